# revision 6
# baseline (speedup 1.0000x reference)
"""Trainium2 Bass kernel for nn_Attention_3375844295015.

RMSNorm -> {Q (normalized), KV (unnormalized)} projections -> RoPE(q,k)
-> causal attention -> out projection, distributed over 8 NeuronCores
Megatron-style: each core owns 2 of the 16 heads (column-shard of
Wq/Wk/Wv, row-shard of Wo) and produces a full-shape partial output;
the host sums the 8 partials (the "all-reduce") and adds bo.

Device dataflow (per core, everything in transposed [feature, seq]
layout so matmuls chain without transposes):
  qT = rope(Wq_c^T @ xT) * r * scale   (r = RMSNorm scale, folded into
                                        the rope cos/sin tables host-side)
  kT = rope(Wk_c^T @ xT)
  v  = xT^T @ Wv_c                     ([seq, dim] layout, + ones column)
  per (batch, head): simT[k, q] = kT^T qT ; e = exp(simT) (no max-sub:
  logits are O(+-10)); causal mask via 0/1 multiply on diagonal chunks;
  A@V with the ones column accumulates both Sum(e*v) and Sum(e) in one
  PSUM group; out = Sum(e*v) / Sum(e); partial = outT^T @ Wo_c.
"""

import os
import sys

sys.path.insert(0, "/opt/trn_rl_repo")

import numpy as np
import ml_dtypes

import concourse.bass as bass
import concourse.mybir as mybir
import concourse.tile as tile_mod
from concourse.bass_utils import run_bass_kernel_spmd
from concourse.vector_clock import ScopedClock

BF16 = ml_dtypes.bfloat16
F32 = mybir.dt.float32
BF = mybir.dt.bfloat16

B, N, D = 2, 2048, 1024
H, DH = 16, 64
INNER = H * DH
EPS = 1e-8
SCALE = DH ** -0.5
NCORES = 8
BN = B * N              # 4096 tokens, col index = b*N + n
KC = 128                # k-position chunk
QT = 512                # q-tile width
NKC_B = N // KC         # 16 k-chunks per batch
NQT_B = N // QT         # 4 q-tiles per batch


def _patched_drain_and_barrier(self, tick_clock, wait_clock):
    # The stock TileContext drain carries one sem-wait per outstanding
    # logical processor; this neuronxcc lowers SP Drain through a CTRL
    # struct that holds fewer waits ("Too many sync wait commands").
    # Put each wait on its own SP NOP ahead of the drain instead.
    nop_inst = self.nc.sync.nop(nofuse=True, hint="pre_drain_waits")
    wait_clock.add_sem_waits(
        nop_inst.ins, ScopedClock({None: tick_clock.global_clock})
    )
    si = nop_inst.ins.sync_info
    waits = list(si.on_wait) if si is not None else []
    if len(waits) > 1:
        si.on_wait = waits[:1]
        for w in waits[1:]:
            extra = self.nc.sync.nop(nofuse=True, hint="pre_drain_waits")
            extra.ins.sync_info = mybir.SyncInfo(on_wait=[w], on_update=[])
    self.nc.sync.drain()
    self.nc.all_engine_barrier()
    popped = self.nc._tile_sem_poison_stack.pop()
    assert popped is self._sem_poison
    self.nc.clear_and_free_semaphores(list(self.sems.allocated().values()))
    self.nc.all_engine_barrier()


tile_mod.TileContext._drain_and_barrier = _patched_drain_and_barrier


def _split_excess_waits(nc, limit=1):
    """walrus CoreV3 lowers at most ~2 sem waits per instruction; move any
    excess onto same-engine NOPs inserted directly before the instruction
    (same-engine program order makes this semantically identical)."""
    ctr = [0]
    for f in nc.m.functions:
        for bb in f.blocks:
            new_insts = []
            for inst in bb.instructions:
                si = inst.sync_info
                lim = 1 if type(inst).__name__ == "InstDrain" else limit
                if si is not None and len(si.on_wait) > lim:
                    waits = list(si.on_wait)
                    si.on_wait = waits[-lim:]
                    extra = waits[:-lim]
                    for i in range(0, len(extra), limit):
                        ctr[0] += 1
                        nop = mybir.InstNoOp(
                            name=f"WSPLIT-{ctr[0]}",
                            engine=inst.engine,
                            bass_nofuse=True,
                            sync_info=mybir.SyncInfo(
                                on_wait=extra[i:i + limit], on_update=[]
                            ),
                        )
                        new_insts.append(nop)
                new_insts.append(inst)
            bb.instructions[:] = new_insts
    return ctr[0]


def _build_program():
    nc = bass.Bass()
    dt = mybir.dt

    xt_d = nc.declare_dram_parameter("xt", [8, 128, BN], dt.bfloat16, isOutput=False)
    wq_d = nc.declare_dram_parameter("wq", [128, 1024], dt.bfloat16, isOutput=False)
    wk_d = nc.declare_dram_parameter("wk", [128, 1024], dt.bfloat16, isOutput=False)
    wv_d = nc.declare_dram_parameter("wv", [128, 1024], dt.bfloat16, isOutput=False)
    wo_d = nc.declare_dram_parameter("wo", [128, 1024], dt.bfloat16, isOutput=False)
    cosq_d = nc.declare_dram_parameter("cosq", [128, BN], dt.bfloat16, isOutput=False)
    sinq_d = nc.declare_dram_parameter("sinq", [128, BN], dt.bfloat16, isOutput=False)
    cosk_d = nc.declare_dram_parameter("cosk", [128, N], dt.bfloat16, isOutput=False)
    sink_d = nc.declare_dram_parameter("sink", [128, N], dt.bfloat16, isOutput=False)
    rot_d = nc.declare_dram_parameter("rot", [128, 128], dt.bfloat16, isOutput=False)
    mask_d = nc.declare_dram_parameter("mask", [128, 4, QT], dt.bfloat16, isOutput=False)
    out_d = nc.declare_dram_parameter("out", [BN, D], dt.bfloat16, isOutput=True)

    rec_scr = nc.dram_tensor("rec_scr", [B * NQT_B * 2, QT], dt.float32)

    from contextlib import ExitStack

    with tile_mod.TileContext(nc) as tc, ExitStack() as ctx:
        consts = ctx.enter_context(tc.tile_pool(name="consts", bufs=1))
        sbuf = ctx.enter_context(tc.tile_pool(name="sbuf", bufs=1))
        work = ctx.enter_context(tc.tile_pool(name="work", bufs=3))
        epool = ctx.enter_context(tc.tile_pool(name="epool", bufs=3))
        rpool = ctx.enter_context(tc.tile_pool(name="rpool", bufs=4))
        ps_gen = ctx.enter_context(tc.tile_pool(name="ps_gen", bufs=2, space="PSUM"))
        ps_sim = ctx.enter_context(tc.tile_pool(name="ps_sim", bufs=2, space="PSUM"))
        ps_av = ctx.enter_context(tc.tile_pool(name="ps_av", bufs=2, space="PSUM"))

        # ---- constants / inputs resident in SBUF ----
        xt_sb = consts.tile([128, 8, BN], BF, tag="xt")
        for kc in range(8):
            nc.sync.dma_start(xt_sb[:, kc, :], xt_d[kc])
        wq_sb = consts.tile([128, 8, 128], BF, tag="wq")
        nc.sync.dma_start(wq_sb[:], wq_d[:].rearrange("p (k m) -> p k m", k=8))
        wk_sb = consts.tile([128, 8, 128], BF, tag="wk")
        nc.sync.dma_start(wk_sb[:], wk_d[:].rearrange("p (k m) -> p k m", k=8))
        wv_sb = consts.tile([128, 8, 128], BF, tag="wv")
        nc.sync.dma_start(wv_sb[:], wv_d[:].rearrange("p (k m) -> p k m", k=8))
        wo_sb = consts.tile([128, 1024], BF, tag="wo")
        nc.sync.dma_start(wo_sb[:], wo_d[:])
        cosq_sb = consts.tile([128, BN], BF, tag="cosq")
        nc.sync.dma_start(cosq_sb[:], cosq_d[:])
        sinq_sb = consts.tile([128, BN], BF, tag="sinq")
        nc.sync.dma_start(sinq_sb[:], sinq_d[:])
        cosk_sb = consts.tile([128, N], BF, tag="cosk")
        nc.sync.dma_start(cosk_sb[:], cosk_d[:])
        sink_sb = consts.tile([128, N], BF, tag="sink")
        nc.sync.dma_start(sink_sb[:], sink_d[:])
        rot_sb = consts.tile([128, 128], BF, tag="rot")
        nc.sync.dma_start(rot_sb[:], rot_d[:])
        mask_sb = consts.tile([128, 4, QT], BF, tag="mask")
        nc.sync.dma_start(mask_sb[:], mask_d[:])

        # ---- persistent intermediates ----
        qT = sbuf.tile([128, BN], BF, tag="qT")       # 2 heads stacked [64|64]
        kT = sbuf.tile([128, BN], BF, tag="kT")
        v0 = sbuf.tile([128, 32, 65], BF, tag="v0")   # [kpos, chunk, dim+1]
        v1 = sbuf.tile([128, 32, 65], BF, tag="v1")
        outT = sbuf.tile([128, BN], BF, tag="outT")   # unused sums excluded
        nc.vector.memset(v0[:], 1.0)
        nc.vector.memset(v1[:], 1.0)

        # ---- phase 1: q/k projections + rope ----
        def proj_rope(w_sb, cos_sb, sin_sb, dst, j, trig_cols):
            # one column tile j of 512 tokens
            cols = slice(j * QT, (j + 1) * QT)
            ps = ps_gen.tile([128, QT], F32, tag="gen")
            for kc in range(8):
                nc.tensor.matmul(
                    ps[:],
                    w_sb[:, kc, :],
                    xt_sb[:, kc, cols],
                    start=(kc == 0),
                    stop=(kc == 7),
                )
            raw = work.tile([128, QT], BF, tag="raw")
            nc.vector.tensor_copy(raw[:], ps[:])
            psr = ps_gen.tile([128, QT], F32, tag="gen")
            nc.tensor.matmul(psr[:], rot_sb[:], raw[:], start=True, stop=True)
            t1 = work.tile([128, QT], BF, tag="t1")
            nc.vector.tensor_mul(t1[:], raw[:], cos_sb[:, trig_cols])
            t2 = work.tile([128, QT], BF, tag="t2")
            nc.vector.tensor_mul(t2[:], psr[:], sin_sb[:, trig_cols])
            nc.gpsimd.tensor_add(dst[:, cols], t1[:], t2[:])

        for j in range(BN // QT):
            trig_cols = slice(j * QT, (j + 1) * QT)
            proj_rope(wq_sb, cosq_sb, sinq_sb, qT, j, trig_cols)
        for j in range(BN // QT):
            # k trig tables cover one batch of N columns
            jq = j % (N // QT)
            trig_cols = slice(jq * QT, (jq + 1) * QT)
            proj_rope(wk_sb, cosk_sb, sink_sb, kT, j, trig_cols)

        # ---- phase 2: v projection ([seq, dim] layout + ones col) ----
        for rc in range(32):
            rows = slice(rc * 128, (rc + 1) * 128)
            ps = ps_gen.tile([128, 128], F32, tag="gen")
            for kc in range(8):
                nc.tensor.matmul(
                    ps[:],
                    xt_sb[:, kc, rows],
                    wv_sb[:, kc, :],
                    start=(kc == 0),
                    stop=(kc == 7),
                )
            nc.vector.tensor_copy(v0[:, rc, 0:64], ps[:, 0:64])
            nc.vector.tensor_copy(v1[:, rc, 0:64], ps[:, 64:128])

        # ---- phase 3: attention ----
        for b in range(B):
            base = b * N
            for t in range(NQT_B):
                qcols = slice(base + t * QT, base + (t + 1) * QT)
                pav = [
                    ps_av.tile([65, QT], F32, tag="av", name=f"pav{hh}")
                    for hh in range(2)
                ]
                nkc = 4 * (t + 1)
                for kcp in range(nkc // 2):
                    kc0 = 2 * kcp
                    for h in range(2):
                        hp = slice(64 * h, 64 * h + 64)
                        sim = ps_sim.tile([128, 2 * QT], F32, tag="sim")
                        for u in range(2):
                            kc = kc0 + u
                            kcols = slice(base + kc * KC, base + (kc + 1) * KC)
                            nc.tensor.matmul(
                                sim[:, u * QT:(u + 1) * QT],
                                kT[hp, kcols],
                                qT[hp, qcols],
                                start=True,
                                stop=True,
                            )
                        e = epool.tile([128, 2 * QT], BF, tag=f"e{h}")
                        nc.scalar.activation(
                            e[:], sim[:], mybir.ActivationFunctionType.Exp
                        )
                        for u in range(2):
                            kc = kc0 + u
                            off = kc * KC - t * QT
                            if off >= 0:
                                nc.gpsimd.tensor_mul(
                                    e[:, u * QT:(u + 1) * QT],
                                    e[:, u * QT:(u + 1) * QT],
                                    mask_sb[:, off // KC, :],
                                )
                        vsb = v0 if h == 0 else v1
                        for u in range(2):
                            kc = kc0 + u
                            nc.tensor.matmul(
                                pav[h][:],
                                vsb[:, b * 16 + kc, :],
                                e[:, u * QT:(u + 1) * QT],
                                start=(kc == 0),
                                stop=(kc == nkc - 1),
                            )
                for h in range(2):
                    hp = slice(64 * h, 64 * h + 64)
                    ridx = (b * NQT_B + t) * 2 + h
                    rec = rpool.tile([1, QT], F32, tag="rec")
                    nc.vector.reciprocal(rec[:], pav[h][64:65, :])
                    nc.sync.dma_start(rec_scr[ridx:ridx + 1, :], rec[:])
                    recb = rpool.tile([64, QT], F32, tag="recb")
                    nc.sync.dma_start(
                        recb[:], rec_scr[ridx:ridx + 1, :].to_broadcast((64, QT))
                    )
                    nc.vector.tensor_mul(outT[hp, qcols], pav[h][0:64, :], recb[:])

        # ---- phase 4: out projection (partial = outT^T @ Wo_c) ----
        for m in range(BN // 128):
            mrows = slice(m * 128, (m + 1) * 128)
            for nn_ in range(2):
                ncols = slice(nn_ * 512, (nn_ + 1) * 512)
                ps = ps_gen.tile([128, 512], F32, tag="gen")
                nc.tensor.matmul(
                    ps[:], outT[:, mrows], wo_sb[:, ncols], start=True, stop=True
                )
                ot = work.tile([128, 512], BF, tag="ot")
                nc.vector.tensor_copy(ot[:], ps[:])
                nc.sync.dma_start(out_d[mrows, ncols], ot[:])

    _split_excess_waits(nc)
    return nc


_PROGRAM = None


def _get_program():
    global _PROGRAM
    if _PROGRAM is None:
        _PROGRAM = _build_program()
    return _PROGRAM


def _host_prep(x, pos_emb, gamma, Wq, Wkv, Wo):
    """Build the per-core input maps."""
    xf = np.ascontiguousarray(x.reshape(BN, D))
    xT = np.ascontiguousarray(xf.T).astype(BF16)        # [1024, 4096]
    xt = np.ascontiguousarray(xT.reshape(8, 128, BN))

    r = 1.0 / np.maximum(
        np.linalg.norm(xf.astype(np.float64), axis=1).astype(np.float32)
        * (D ** -0.5),
        EPS,
    )

    fr = pos_emb[0, 0, :, :32].astype(np.float32)        # [N, 32]
    cos_t = np.cos(fr).T                                 # [32, N]
    sin_t = np.sin(fr).T
    cos128 = np.tile(cos_t, (4, 1)).astype(np.float32)   # [128, N]
    sin128 = np.tile(sin_t, (4, 1)).astype(np.float32)
    colpos = np.arange(BN) % N
    rq = (r * SCALE).astype(np.float32)
    cosq = (cos128[:, colpos] * rq[None, :]).astype(BF16)
    sinq = (sin128[:, colpos] * rq[None, :]).astype(BF16)
    cosk = cos128.astype(BF16)
    sink = sin128.astype(BF16)

    R = np.zeros((128, 128), np.float32)
    for bb in (0, 64):
        for i in range(32):
            R[bb + i + 32, bb + i] = -1.0
            R[bb + i, bb + i + 32] = 1.0
    R = R.astype(BF16)

    p = np.arange(128)[:, None]
    cq = np.arange(QT)[None, :]
    mask = np.stack(
        [((off + p) <= cq).astype(BF16) for off in (0, 128, 256, 384)], axis=1
    )                                                    # [128, 4, 512]

    Wq_s = (gamma[:, None].astype(np.float32) * Wq).astype(BF16)
    Wk = Wkv[:, :INNER].astype(BF16)
    Wv = Wkv[:, INNER:].astype(BF16)
    Wo_b = Wo.astype(BF16)

    def warrange(w):  # [1024, 128] -> [128, 1024] with [p, kc*128+m]
        return np.ascontiguousarray(
            w.reshape(8, 128, 128).transpose(1, 0, 2).reshape(128, 1024)
        )

    in_maps = []
    for c in range(NCORES):
        sl = slice(128 * c, 128 * (c + 1))
        in_maps.append(
            {
                "xt": xt,
                "wq": warrange(Wq_s[:, sl]),
                "wk": warrange(Wk[:, sl]),
                "wv": warrange(Wv[:, sl]),
                "wo": np.ascontiguousarray(Wo_b[sl, :]),
                "cosq": cosq,
                "sinq": sinq,
                "cosk": cosk,
                "sink": sink,
                "rot": R,
                "mask": mask,
            }
        )
    return in_maps


def run(inputs, trace=False, trace_kwargs=None):
    nc = _get_program()
    in_maps = _host_prep(
        np.asarray(inputs["x"]),
        np.asarray(inputs["pos_emb"]),
        np.asarray(inputs["gamma"]),
        np.asarray(inputs["Wq"]),
        np.asarray(inputs["Wkv"]),
        np.asarray(inputs["Wo"]),
    )
    res = run_bass_kernel_spmd(
        nc,
        in_maps,
        list(range(NCORES)),
        trace=trace,
        trace_kwargs=trace_kwargs or {},
    )
    out = np.zeros((BN, D), np.float32)
    for c in range(NCORES):
        out += res.results[c]["out"].astype(np.float32)
    out += np.asarray(inputs["bo"]).astype(np.float32)[None, :]
    out = out.reshape(B, N, D).astype(np.float32)
    return out, res


def kernel(**inputs):
    out, _ = run(inputs, trace=False)
    return out
